# revision 63
# baseline (speedup 1.0000x reference)
"""Two-layer GCN (PyG GCNConv semantics) on 8 Trainium2 NeuronCores via Bass.

Nodes are partitioned across the 8 cores by destination (graph parallel).
Both layers are computed aggregate-then-transform, so the gather table for a
layer is just dinv * X (features scaled by the symmetric normalization).
Message gathering uses the SWDGE dma_gather primitive (int16 indices, 256B
rows).

Layer 1 is dst-sharded: the int16 index range is handled by splitting the
node table into 4 src buckets with a degree-sorted ELL per (core, bucket);
per-bucket partials are un-permuted and summed with combine gathers.  The
whole layer is pipelined: deg -> dinv -> table build -> edge gathers run
bucket-by-bucket (bucket 0 gathers start while bucket 1's table is being
built), reduce outputs are dumped per K-run, combines of bucket b overlap
bucket b+1's gathers, and the transform consumes the accumulator in mm-group
slices.

Layer 2 is src-sharded, which removes the inter-layer AllGather entirely:
each core gathers from its OWN t2shard immediately after its layer-1
transform, reduces per sorted-dst chunk, dma_scatter_adds the partial dst
sums into a p-major slab (4 int16 dst-range buckets), and a single cheap
ReduceScatter hands every core its summed dst shard.  The slab is bf16 with
a parity-split sort packing even/odd dst nodes into shared 256B scatter
rows, halving scatter descriptor size, slab zero-init, and RS payload.  Scatter desc-gen is
lagged two groups behind the gathers so it never head-of-line blocks the
Pool SEQ; ELL staging is prefetched one bucket ahead.
"""
import numpy as np
import ml_dtypes

import concourse.bass as bass
import concourse.bacc as bacc
import concourse.mybir as mybir
import concourse.tile as tile
from concourse import library_config
from concourse.bass_utils import run_bass_kernel_spmd
from concourse.masks import make_identity

NC = 8
CH = 128
NBUCK = 4
CPB = NC // NBUCK   # cores (shards) per bucket
TILE_COLS = 80      # target dest columns per aggregation dma_gather call
GCAP = 16           # max chunks per agg group (bounds reduce-out tile)
GCAP2 = 24          # max chunks per layer-2 group
SUPER_COLS = 1      # max columns per merged gather call (1 = one K-run per gather)
DEG_CAP = 3072      # max elems/partition per deg tile (bf16)
MMG = 7             # chunks per matmul group (98 = 14 uniform groups)


def _split_multi_waits(nc, max_waits=1):
    """walrus workaround: only ONE sync-wait per CTRL/pseudo-DMA instruction;
    hoist extra waits into single-wait nops on the same engine."""
    n = 0
    for f in nc.m.functions:
        for bb in f.blocks:
            new_insts = []
            for inst in bb.instructions:
                si = inst.sync_info
                if si is not None and si.on_wait and len(si.on_wait) > max_waits:
                    waits = list(si.on_wait)
                    for w in waits[max_waits:]:
                        n += 1
                        new_insts.append(mybir.InstNoOp(
                            name=f"WSPLIT-{n}-{inst.name}",
                            sync_info=mybir.SyncInfo(on_wait=[w], on_update=[]),
                            bass_nofuse=True,
                            engine=inst.engine,
                        ))
                    si.on_wait = waits[:max_waits]
                new_insts.append(inst)
            bb.instructions.clear()
            for i in new_insts:
                bb.add_instruction(i)
    return n


def _wrap_idx(lst):
    """int16 list -> dma_gather wrapped layout [128, len//16]."""
    w = lst.reshape(-1, 16).T
    return np.ascontiguousarray(np.tile(w, (8, 1)))


def _cumcount(key, n):
    o = np.argsort(key, kind="stable")
    sk = key[o]
    starts = np.flatnonzero(np.r_[True, sk[1:] != sk[:-1]])
    lens = np.diff(np.r_[starts, n])
    kk = np.arange(n, dtype=np.int64) - np.repeat(starts, lens)
    k = np.empty(n, dtype=np.int64)
    k[o] = kk
    return k


def _runs_capped(K, cap, gcap=None):
    """Groups of consecutive equal-K chunks with g*K<=cap (g>=1), skip K==0."""
    groups = []
    q, n = 0, len(K)
    while q < n:
        k = int(K[q])
        if k == 0:
            q += 1
            continue
        g = 1
        lim = max(1, cap // k)
        if gcap is not None:
            lim = min(lim, gcap)
        while q + g < n and int(K[q + g]) == k and g < lim:
            g += 1
        groups.append((q, g, k))
        q += g
    return groups


def _prep(x, edge_index, edge_weights, W1, b1, W2, b2):
    N, DIN = x.shape
    DH = W1.shape[0]
    DOUT = W2.shape[0]
    E = edge_index.shape[1]
    assert DIN == DH, "partial buffers assume DIN == DH"
    per_core = -(-N // NC)
    SHARD = -(-per_core // CH) * CH
    NTAB = NC * SHARD
    BUCK = NTAB // NBUCK
    NCHUNK = SHARD // CH
    assert BUCK <= 32767 and NC % NBUCK == 0

    src = np.asarray(edge_index[0], dtype=np.int64)
    dst = np.asarray(edge_index[1], dtype=np.int64)
    w = np.asarray(edge_weights, dtype=np.float32)

    indeg = np.bincount(dst, minlength=N)
    order = np.argsort(-indeg, kind="stable")
    i_of = np.empty(N, dtype=np.int64)
    i_of[order] = np.arange(N)
    tpos = (i_of % NC) * SHARD + (i_of // NC)  # node -> table row

    x_perm = np.zeros((NTAB, DIN), dtype=np.float32)
    x_perm[tpos] = np.asarray(x, dtype=np.float32)
    # per-core own rows in p-major layout [128, NCHUNK*DIN]
    x_own_pm = (x_perm.reshape(NC, NCHUNK, CH, DIN).transpose(0, 2, 1, 3)
                .reshape(NC, CH, NCHUNK * DIN))

    tsrc = tpos[src]
    tdst = tpos[dst]
    ec = tdst // SHARD
    ep = tdst % SHARD
    eb = tsrc // BUCK
    eloc = (tsrc % BUCK).astype(np.int16)            # layer-1 (natural rows)
    # layer 2 is src-sharded: the executing core owns the edge's SOURCE and
    # gathers from its own t2shard (p-major rows, unit = p*NCHUNK + q).
    c2 = tsrc // SHARD
    ps_ = (tsrc % SHARD) % CH
    qs_ = (tsrc % SHARD) // CH
    eloc2 = (ps_ * NCHUNK + qs_).astype(np.int16)

    # per (core, bucket) in-slot counts and degree-sorted permutations
    cb = ec * NBUCK + eb
    cnt = np.bincount(cb * SHARD + ep, minlength=NC * NBUCK * SHARD)
    cnt = cnt.reshape(NC, NBUCK, SHARD)
    pi = np.argsort(-cnt, axis=2, kind="stable")
    inv = np.empty_like(pi)
    np.put_along_axis(inv, pi, np.broadcast_to(np.arange(SHARD), pi.shape).copy(), axis=2)
    cnt_sorted = np.take_along_axis(cnt, pi, axis=2)
    Kb = np.maximum(cnt_sorted[:, :, ::CH].max(axis=0), 1)   # [NBUCK, NCHUNK]
    assert int(Kb.max()) <= TILE_COLS
    colsb = Kb.sum(axis=1)
    bucket_base = np.concatenate([[0], np.cumsum(colsb)]).astype(np.int64)
    TCOLS = int(bucket_base[-1])
    CB_MAX = int(colsb.max())
    coloff = np.zeros((NBUCK, NCHUNK), dtype=np.int64)
    for b in range(NBUCK):
        coloff[b] = bucket_base[b] + np.r_[0, np.cumsum(Kb[b])[:-1]]

    # per-edge slot
    rp = inv[ec, eb, ep]
    k = _cumcount(cb * SHARD + rp, E)
    col = coloff[eb, rp // CH] + k
    slot = col * CH + (rp % CH)
    ell_idx = np.zeros((NC, TCOLS * CH), dtype=np.int16)
    ell_w = np.zeros((NC, TCOLS * CH), dtype=np.float32)
    ell_idx[ec, slot] = eloc
    ell_w[ec, slot] = w

    ell_idx_w = np.stack([_wrap_idx(ell_idx[c]) for c in range(NC)])
    ell_w_sb = np.ascontiguousarray(
        ell_w.reshape(NC, TCOLS, CH).transpose(0, 2, 1))

    def _supers(runs, cap):
        """Pack consecutive K-runs into one gather call of <= cap columns."""
        supers = []
        cur, cur_cols, c0 = [], 0, None
        for (q0, g, K, cs) in runs:
            cols = g * K
            if cur and cur_cols + cols > cap:
                supers.append((c0, cur_cols, tuple(cur)))
                cur, cur_cols = [], 0
            if not cur:
                c0 = cs
            cur.append((q0, g, K, cs))
            cur_cols += cols
        if cur:
            supers.append((c0, cur_cols, tuple(cur)))
        return supers

    agg_groups = [[] for _ in range(NBUCK)]
    for b in range(NBUCK):
        runs = [(q0, g, K, int(coloff[b, q0]))
                for (q0, g, K) in _runs_capped(Kb[b], TILE_COLS, GCAP)]
        agg_groups[b] = _supers(runs, SUPER_COLS)

    # ---- layer-2 src-sharded ELL: dst positions in p-major table order ----
    # partial2 row (c*128+p)*NCHUNK + q == dst table row within core c's shard
    # at partition p, chunk q.  Buckets are contiguous p-major ranges so the
    # scatter-add offsets fit int16.
    PB = NTAB // NBUCK
    NCK2 = PB // CH                       # sorted chunks per bucket
    pos2 = ec * SHARD + (ep % CH) * NCHUNK + ep // CH
    eb2 = pos2 // PB
    lp2 = pos2 % PB
    cnt2 = np.bincount(c2 * NTAB + pos2, minlength=NC * NTAB)
    cnt2 = cnt2.reshape(NC, NBUCK, PB)
    # parity-split sort: even local positions first, then odd, each sorted by
    # count.  A bf16 scatter row then packs nodes (2r, 2r+1) into one 256B
    # stride, and every K-run lies wholly in one parity region.
    pi2 = np.concatenate(
        [2 * np.argsort(-cnt2[:, :, 0::2], axis=2, kind="stable"),
         2 * np.argsort(-cnt2[:, :, 1::2], axis=2, kind="stable") + 1], axis=2)
    inv2 = np.empty_like(pi2)
    np.put_along_axis(inv2, pi2,
                      np.broadcast_to(np.arange(PB), pi2.shape).copy(), axis=2)
    cnt2_sorted = np.take_along_axis(cnt2, pi2, axis=2)
    K2 = cnt2_sorted[:, :, ::CH].max(axis=0)          # [NBUCK, NCK2]
    cols2b = K2.sum(axis=1)
    base2 = np.concatenate([[0], np.cumsum(cols2b)]).astype(np.int64)
    TCOLS2 = int(base2[-1])
    CB2_MAX = int(cols2b.max())
    coloff2 = np.zeros((NBUCK, NCK2), dtype=np.int64)
    for b in range(NBUCK):
        coloff2[b] = base2[b] + np.r_[0, np.cumsum(K2[b])[:-1]]

    rp2 = inv2[c2, eb2, lp2]
    k2e = _cumcount((c2 * NBUCK + eb2) * PB + rp2, E)
    col2 = coloff2[eb2, rp2 // CH] + k2e
    slot2 = col2 * CH + (rp2 % CH)
    ell2_idx = np.zeros((NC, TCOLS2 * CH), dtype=np.int16)
    ell2_w = np.zeros((NC, TCOLS2 * CH), dtype=np.float32)
    ell2_idx[c2, slot2] = eloc2
    ell2_w[c2, slot2] = w
    ell2_idx_w = np.stack([_wrap_idx(ell2_idx[c]) for c in range(NC)])
    ell2_w_sb = np.ascontiguousarray(
        ell2_w.reshape(NC, TCOLS2, CH).transpose(0, 2, 1))

    agg2_groups = [[] for _ in range(NBUCK)]
    zero2_groups = [[] for _ in range(NBUCK)]
    HCK = NCK2 // 2   # chunks per parity region (even: [0,HCK), odd: rest)
    for b in range(NBUCK):
        runs = [(q0, g, K, int(coloff2[b, q0]))
                for (q0, g, K) in _runs_capped(K2[b][:HCK], TILE_COLS, GCAP2)]
        runs += [(q0 + HCK, g, K, int(coloff2[b, q0 + HCK]))
                 for (q0, g, K) in _runs_capped(K2[b][HCK:], TILE_COLS, GCAP2)]
        agg2_groups[b] = _supers(runs, SUPER_COLS)
        # sorted chunks with no edges anywhere: scatter zeros so the RS slab
        # needs no separate zero-init pass
        zq = np.flatnonzero(K2[b] == 0)
        i = 0
        while i < len(zq):
            j = i
            while (j + 1 < len(zq) and zq[j + 1] == zq[j] + 1
                   and zq[j + 1] - zq[i] + 1 <= GCAP2):
                j += 1
            zero2_groups[b].append((int(zq[i]), int(zq[j] - zq[i] + 1)))
            i = j + 1

    # scatter indices: sorted slot s -> natural p-major offset within bucket
    scat = np.empty((NC, CH, NBUCK * (PB // 16)), dtype=np.int16)
    for c in range(NC):
        for b in range(NBUCK):
            scat[c, :, b * (PB // 16):(b + 1) * (PB // 16)] = _wrap_idx(
                (pi2[c, b] // 2).astype(np.int16))

    # combine permutation indices (values address p-major partial layout)
    comb = np.empty((NC, CH, NBUCK * (SHARD // 16)), dtype=np.int16)
    for c in range(NC):
        for b in range(NBUCK):
            r = inv[c, b]
            vals = ((r % CH) * NCHUNK + (r // CH)).astype(np.int16)
            comb[c, :, b * (SHARD // 16):(b + 1) * (SHARD // 16)] = _wrap_idx(vals)

    # full-table deg ELL in build-tile layout, bucket-aligned tiles:
    # build tile T covers rows [T*TSZ,(T+1)*TSZ); partition p holds rows
    # T*TSZ + p*VS + s for s in [0,VS)
    TSZ = CH
    for kdiv in range(32, 0, -1):
        if BUCK % (CH * kdiv) == 0:
            TSZ = CH * kdiv
            break
    VS = TSZ // CH
    NT = NTAB // TSZ
    TPB = NT // NBUCK   # build tiles per bucket
    indeg_tab = np.zeros(NTAB, dtype=np.int64)
    indeg_tab[tpos] = indeg
    KT = indeg_tab.reshape(NT, TSZ).max(axis=1)
    dega_groups = [[] for _ in range(NBUCK)]   # per bucket: (T0, g, K, off)
    off = 0
    meta_T = np.zeros((NT, 3), dtype=np.int64)
    for b in range(NBUCK):
        for (Tl, g, K) in _runs_capped(KT[b * TPB:(b + 1) * TPB],
                                       max(DEG_CAP // VS, 1)):
            T0 = b * TPB + Tl
            dega_groups[b].append((T0, g, K, off))
            meta_T[T0:T0 + g] = (off, T0, g * VS * K)
            off += CH * g * VS * K
    LDEG = max(off, 16)
    wdeg = np.zeros(LDEG, dtype=np.float32)
    k2 = _cumcount(tdst, E)
    T_e = tdst // TSZ
    p_e = (tdst % TSZ) // VS
    s_e = tdst % VS
    pos = (meta_T[T_e, 0] + p_e * meta_T[T_e, 2]
           + ((T_e - meta_T[T_e, 1]) * VS + s_e) * KT[T_e] + k2)
    wdeg[pos] = w

    # per-core own-block deg ELL (chunk-p layout, shared K across cores)
    KQ = np.maximum(indeg_tab.reshape(NC, NCHUNK, CH).max(axis=(0, 2)), 0)
    degown_groups = []
    offo = 0
    meta_q = np.zeros((NCHUNK, 3), dtype=np.int64)
    for (q0, g, K) in _runs_capped(KQ, DEG_CAP):
        degown_groups.append((q0, g, K, offo))
        meta_q[q0:q0 + g] = (offo, q0, g * K)
        offo += CH * g * K
    LDEGO = max(offo, 16)
    wdeg_own = np.zeros((NC, LDEGO), dtype=np.float32)
    q_e = ep // CH
    pos_o = (meta_q[q_e, 0] + (ep % CH) * meta_q[q_e, 2]
             + (q_e - meta_q[q_e, 1]) * KQ[q_e] + k2)
    wdeg_own[ec, pos_o] = w

    meta = dict(
        N=N, E=E, DIN=DIN, DH=DH, DOUT=DOUT,
        SHARD=SHARD, NTAB=NTAB, BUCK=BUCK, NCHUNK=NCHUNK,
        TCOLS=TCOLS, LDEG=LDEG, LDEGO=LDEGO, CB_MAX=CB_MAX,
        TSZ=TSZ, VS=VS, NT=NT, TPB=TPB,
        PB=PB, NCK2=NCK2, TCOLS2=TCOLS2, CB2_MAX=CB2_MAX,
        bucket_base=tuple(int(v) for v in bucket_base),
        colsb=tuple(int(v) for v in colsb),
        base2=tuple(int(v) for v in base2),
        cols2b=tuple(int(v) for v in cols2b),
        agg_groups=tuple(tuple(g) for g in agg_groups),
        agg2_groups=tuple(tuple(g) for g in agg2_groups),
        zero2_groups=tuple(tuple(g) for g in zero2_groups),
        dega_groups=tuple(tuple(g) for g in dega_groups),
        degown_groups=tuple(degown_groups),
    )

    in_maps = []
    for c in range(NC):
        in_maps.append({
            "x_perm": x_perm,
            "x_own": np.ascontiguousarray(x_own_pm[c]),
            "wdeg": wdeg.reshape(1, -1).astype(ml_dtypes.bfloat16),
            "wdeg_own": wdeg_own[c].reshape(1, -1).astype(ml_dtypes.bfloat16),
            "ell_idx": ell_idx_w[c],
            "ell_idx2": ell2_idx_w[c],
            "ell_w": ell_w_sb[c],
            "ell_w2": ell2_w_sb[c],
            "scat_idx": scat[c],
            "comb_idx": comb[c],
            "W1T": np.ascontiguousarray(np.asarray(W1, np.float32).T),
            "W2T": np.ascontiguousarray(np.asarray(W2, np.float32).T),
            "b1bc": np.broadcast_to(np.asarray(b1, np.float32), (CH, DH)).copy(),
            "b2bc": np.broadcast_to(np.asarray(b2, np.float32), (CH, DOUT)).copy(),
        })
    return meta, in_maps, tpos


def _build(meta):
    SHARD = meta["SHARD"]; NTAB = meta["NTAB"]; BUCK = meta["BUCK"]
    NCHUNK = meta["NCHUNK"]; TCOLS = meta["TCOLS"]; CB_MAX = meta["CB_MAX"]
    DIN = meta["DIN"]; DH = meta["DH"]; DOUT = meta["DOUT"]
    TSZ = meta["TSZ"]; VS = meta["VS"]; TPB = meta["TPB"]
    PB = meta["PB"]; TCOLS2 = meta["TCOLS2"]; CB2_MAX = meta["CB2_MAX"]
    BB = meta["bucket_base"]; BB2 = meta["base2"]
    HSH = SHARD // 2            # half-shard rows for combine gathers
    HCI = SHARD // 16 // 2      # comb idx cols per half
    CBW = max(CB_MAX, CB2_MAX)  # staged weight tile cols (shared L1/L2)

    nc = bacc.Bacc(None, debug=True)
    f32, i16, bf16 = mybir.dt.float32, mybir.dt.int16, mybir.dt.bfloat16

    x_perm = nc.dram_tensor("x_perm", [NTAB, DIN], f32, kind="ExternalInput")
    x_own = nc.dram_tensor("x_own", [CH, NCHUNK * DIN], f32, kind="ExternalInput")
    wdeg = nc.dram_tensor("wdeg", [1, meta["LDEG"]], bf16, kind="ExternalInput")
    wdeg_own = nc.dram_tensor("wdeg_own", [1, meta["LDEGO"]], bf16, kind="ExternalInput")
    ell_idx = nc.dram_tensor("ell_idx", [CH, TCOLS * 8], i16, kind="ExternalInput")
    ell_idx2 = nc.dram_tensor("ell_idx2", [CH, TCOLS2 * 8], i16, kind="ExternalInput")
    ell_w = nc.dram_tensor("ell_w", [CH, TCOLS], f32, kind="ExternalInput")
    ell_w2 = nc.dram_tensor("ell_w2", [CH, TCOLS2], f32, kind="ExternalInput")
    scat_idx = nc.dram_tensor("scat_idx", [CH, NBUCK * (PB // 16)], i16, kind="ExternalInput")
    comb_idx = nc.dram_tensor("comb_idx", [CH, NBUCK * (SHARD // 16)], i16, kind="ExternalInput")
    W1T = nc.dram_tensor("W1T", [DIN, DH], f32, kind="ExternalInput")
    W2T = nc.dram_tensor("W2T", [DH, DOUT], f32, kind="ExternalInput")
    b1bc = nc.dram_tensor("b1bc", [CH, DH], f32, kind="ExternalInput")
    b2bc = nc.dram_tensor("b2bc", [CH, DOUT], f32, kind="ExternalInput")
    out = nc.dram_tensor("out", [CH, NCHUNK * DOUT], f32, kind="ExternalOutput")

    mm_groups = [(s, min(MMG, NCHUNK - s)) for s in range(0, NCHUNK, MMG)]

    with tile.TileContext(nc) as tc:
        with (
            tc.tile_pool(name="const", bufs=1) as cpool,
            tc.tile_pool(name="xt", bufs=2) as xtp,
            tc.tile_pool(name="degt", bufs=2) as dpool,
            tc.tile_pool(name="gidx", bufs=2) as ipool,
            tc.tile_pool(name="gd", bufs=3) as gpool,
            tc.tile_pool(name="red", bufs=3) as rpool,
            tc.tile_pool(name="cg", bufs=2) as cgpool,
            tc.tile_pool(name="acc", bufs=1) as apool,
            tc.tile_pool(name="sl", bufs=2) as spool,
            tc.tile_pool(name="mm", bufs=2) as mpool,
            tc.tile_pool(name="tp", bufs=4) as tpool,
            tc.tile_pool(name="psum", bufs=2, space="PSUM") as psum,
            tc.tile_pool(name="psumtp", bufs=4, space="PSUM") as psumtp,
            tc.tile_pool(name="dram", bufs=1, space="DRAM") as dram,
        ):
            nc.gpsimd.load_library(library_config.mlp)

            tables = [dram.tile([BUCK, DIN], f32, tag=f"tab{b}", name=f"tab{b}")
                      for b in range(NBUCK)]
            t2shard = dram.tile([CH, NCHUNK * DH], f32, tag="t2shard")
            partial2 = dram.tile([NC * CH, NCHUNK * DH], bf16, tag="partial2")
            rs_out = dram.tile([CH, NCHUNK * DH], bf16, tag="rs_out")
            partials = {
                b: dram.tile([CH, NCHUNK * DH], f32, tag=f"part1_{b}",
                             name=f"part1_{b}")
                for b in range(NBUCK)
            }

            W1T_t = cpool.tile([DIN, DH], f32, tag="w1t")
            W2T_t = cpool.tile([DH, DOUT], f32, tag="w2t")
            b1_t = cpool.tile([CH, DH], f32, tag="b1")
            b2_t = cpool.tile([CH, DOUT], f32, tag="b2")
            ident = cpool.tile([CH, CH], f32, tag="ident")
            comb_t = cpool.tile([CH, NBUCK * (SHARD // 16)], i16, tag="combt")
            wt_b = [cpool.tile([CH, CBW], f32, tag=f"wt{b}", name=f"wt{b}")
                    for b in range(NBUCK)]
            nc.sync.dma_start(out=W1T_t[:], in_=W1T[:, :])
            nc.sync.dma_start(out=W2T_t[:], in_=W2T[:, :])
            nc.sync.dma_start(out=b1_t[:], in_=b1bc[:, :])
            nc.sync.dma_start(out=b2_t[:], in_=b2bc[:, :])
            nc.sync.dma_start(out=comb_t[:], in_=comb_idx[:, :])
            make_identity(nc, ident[:])

            # zero-init for the layer-2 scatter slab; emitted lazily via the
            # deferred stream so it soaks into layer-1's spare DMA bandwidth
            # instead of crowding the critical deg/table reads at the head.
            ZW = NCHUNK // 2 * DH // 2
            zt = cpool.tile([CH, ZW], bf16, tag="zt")
            nc.vector.memset(zt[:], 0.0)

            def zero_steps():
                steps = []

                def zw(a0, h):
                    nc.sync.dma_start(
                        out=partial2[a0 * CH:(a0 + 1) * CH,
                                     h * ZW:(h + 1) * ZW],
                        in_=zt[:])

                for a0 in range(NC):
                    for h in range(4):
                        steps.append(lambda a_=a0, h_=h: zw(a_, h_))
                return steps

            # ---------------- dinv (own chunk-p layout) --------------------
            deg_a = cpool.tile([CH, meta["NT"] * VS], f32, tag="dega")
            dinv_a = cpool.tile([CH, meta["NT"] * VS], f32, tag="dinva")
            deg_o = cpool.tile([CH, NCHUNK], f32, tag="dego")
            dinv_o = cpool.tile([CH, NCHUNK], f32, tag="dinvo")
            nc.vector.memset(deg_a[:], 0.0)
            nc.vector.memset(deg_o[:], 0.0)
            for (q0, g, K, off) in meta["degown_groups"]:
                m = g * K
                t = dpool.tile([CH, DEG_CAP], bf16, tag="degt")
                nc.sync.dma_start(
                    out=t[:, :m],
                    in_=wdeg_own[0, off:off + CH * m].rearrange("(p m) -> p m", p=CH))
                nc.vector.tensor_reduce(
                    out=deg_o[:, q0:q0 + g],
                    in_=t[:, :m].rearrange("p (g k) -> p g k", g=g),
                    axis=mybir.AxisListType.X, op=mybir.AluOpType.add)
            nc.vector.tensor_scalar_add(out=deg_o[:], in0=deg_o[:], scalar1=1.0)
            nc.scalar.sqrt(out=dinv_o[:], in_=deg_o[:])
            nc.vector.reciprocal(out=dinv_o[:], in_=dinv_o[:])

            # ================= layer 1 (dst-sharded) =================
            Dh = DH
            acc = apool.tile([CH, NCHUNK * Dh], f32, tag="acc")
            pending = []

            NS = 7                       # combine slices per bucket
            SLCH = NCHUNK // NS          # chunks per combine slice
            SLR = SLCH * CH              # rows per combine slice
            SLI = SLR // 16              # wrapped idx cols per slice

            def combine_slice(b, s):
                cg = cgpool.tile([CH, SLCH * Dh], f32, tag="cg")
                nc.gpsimd.dma_gather(
                    out_ap=cg[:].rearrange("p (c d) -> p c d", d=Dh),
                    in_ap=partials[b][:, :].rearrange(
                        "p (q f) -> (p q) f", f=Dh),
                    idxs_ap=comb_t[:, b * (SHARD // 16) + s * SLI:
                                   b * (SHARD // 16) + (s + 1) * SLI],
                    num_idxs=SLR, num_idxs_reg=SLR, elem_size=Dh,
                    single_packet=False)
                sl = acc[:, s * SLCH * Dh:(s + 1) * SLCH * Dh]
                if b == 0:
                    nc.vector.tensor_copy(out=sl, in_=cg[:])
                else:
                    nc.vector.tensor_tensor(out=sl, in0=sl, in1=cg[:],
                                            op=mybir.AluOpType.add)

            def build_steps(b):
                """Fine-grained emitters for bucket b's deg->dinv->table and
                ELL staging, to be interleaved with the previous bucket's
                gather groups so the builds never gate the gather pipeline."""
                steps = []

                def deg_step(T0, g, K, off):
                    m = g * VS * K
                    t = dpool.tile([CH, DEG_CAP], bf16, tag="degt", name="degt")
                    nc.sync.dma_start(
                        out=t[:, :m],
                        in_=wdeg[0, off:off + CH * m].rearrange(
                            "(p m) -> p m", p=CH))
                    nc.vector.tensor_reduce(
                        out=deg_a[:, T0 * VS:(T0 + g) * VS],
                        in_=t[:, :m].rearrange("p (v k) -> p v k", k=K),
                        axis=mybir.AxisListType.X, op=mybir.AluOpType.add)

                def dinv_step():
                    dsl = slice(b * TPB * VS, (b + 1) * TPB * VS)
                    nc.vector.tensor_scalar_add(
                        out=deg_a[:, dsl], in0=deg_a[:, dsl], scalar1=1.0)
                    nc.scalar.sqrt(out=dinv_a[:, dsl], in_=deg_a[:, dsl])
                    nc.vector.reciprocal(out=dinv_a[:, dsl], in_=dinv_a[:, dsl])

                def tile_step(tt):
                    T = b * TPB + tt
                    r0 = tt * TSZ
                    xt = xtp.tile([CH, VS * DIN], f32, tag="xt", name="xt")
                    nc.sync.dma_start(
                        out=xt[:],
                        in_=x_perm[T * TSZ:(T + 1) * TSZ, :].rearrange(
                            "(p s) f -> p (s f)", p=CH))
                    nc.vector.tensor_tensor(
                        out=xt[:].rearrange("p (s f) -> p s f", s=VS),
                        in0=xt[:].rearrange("p (s f) -> p s f", s=VS),
                        in1=dinv_a[:, T * VS:(T + 1) * VS][:, :, None]
                        .to_broadcast([CH, VS, DIN]),
                        op=mybir.AluOpType.mult)
                    nc.sync.dma_start(
                        out=tables[b][r0:r0 + TSZ, :].rearrange(
                            "(p s) f -> p (s f)", p=CH),
                        in_=xt[:])

                def stage_step():
                    cols_b = meta["colsb"][b]
                    it = ipool.tile([CH, CBW * 8], i16, tag="it", name="it")
                    nc.sync.dma_start(
                        out=it[:, :cols_b * 8],
                        in_=ell_idx[:, BB[b] * 8:(BB[b] + cols_b) * 8])
                    nc.sync.dma_start(out=wt_b[b][:, :cols_b],
                                      in_=ell_w[:, BB[b]:BB[b] + cols_b])
                    it_b[b] = it

                steps.append(stage_step)
                for (T0, g, K, off) in meta["dega_groups"][b]:
                    steps.append(lambda a=T0, bg=g, cK=K, do=off: deg_step(a, bg, cK, do))
                steps.append(dinv_step)
                for tt in range(TPB):
                    steps.append(lambda t_=tt: tile_step(t_))
                return steps

            it_b = [None] * NBUCK
            for s in build_steps(0):
                s()
            deferred = []
            for b in range(NBUCK):
                if b + 1 < NBUCK:
                    deferred = build_steps(b + 1)
                if b == 0:
                    deferred = deferred + zero_steps()
                it = it_b[b]
                for gi, (c0, cols, runs) in enumerate(meta["agg_groups"][b]):
                    cl = c0 - BB[b]
                    gd = gpool.tile([CH, TILE_COLS * Dh], f32, tag="gd")
                    nc.gpsimd.dma_gather(
                        out_ap=gd[:, :cols * Dh].rearrange("p (c d) -> p c d", c=cols),
                        in_ap=tables[b][:, :],
                        idxs_ap=it[:, cl * 8:(cl + cols) * 8],
                        num_idxs=CH * cols, num_idxs_reg=CH * cols, elem_size=Dh,
                        single_packet=False)
                    if len(runs) == 1 and runs[0][2] == 1:
                        # K==1: the weight multiply IS the aggregate
                        (q0, g, K, csg) = runs[0]
                        rt = rpool.tile([CH, GCAP2 * Dh], f32, tag="rt")
                        nc.vector.tensor_tensor(
                            out=rt[:, :g * Dh].rearrange("p (c d) -> p c d", c=g),
                            in0=gd[:, :cols * Dh].rearrange("p (c d) -> p c d", c=cols),
                            in1=wt_b[b][:, cl:cl + cols][:, :, None]
                            .to_broadcast([CH, cols, Dh]),
                            op=mybir.AluOpType.mult)
                        nc.sync.dma_start(
                            out=partials[b][:, q0 * Dh:(q0 + g) * Dh],
                            in_=rt[:, :g * Dh])
                    else:
                        nc.vector.tensor_tensor(
                            out=gd[:, :cols * Dh].rearrange("p (c d) -> p c d", c=cols),
                            in0=gd[:, :cols * Dh].rearrange("p (c d) -> p c d", c=cols),
                            in1=wt_b[b][:, cl:cl + cols][:, :, None]
                            .to_broadcast([CH, cols, Dh]),
                            op=mybir.AluOpType.mult)
                        for (q0, g, K, csg) in runs:
                            off = csg - c0
                            rt = rpool.tile([CH, GCAP2 * Dh], f32, tag="rt")
                            nc.vector.tensor_reduce(
                                out=rt[:, :g * Dh].rearrange("p (g d) -> p g d", g=g),
                                in_=gd[:, off * Dh:(off + g * K) * Dh].rearrange(
                                    "p (g k d) -> p g d k", g=g, k=K),
                                axis=mybir.AxisListType.X, op=mybir.AluOpType.add)
                            nc.sync.dma_start(
                                out=partials[b][:, q0 * Dh:(q0 + g) * Dh],
                                in_=rt[:, :g * Dh])
                    if gi >= 1 and pending:
                        pending.pop(0)()
                    for _ in range(4):
                        if deferred:
                            deferred.pop(0)()
                while deferred:
                    deferred.pop(0)()
                while pending:
                    pending.pop(0)()
                pending = [lambda bb_=b, s_=s: combine_slice(bb_, s_)
                           for s in range(NS)]
            while pending:
                pending.pop(0)()

            # -------- layer-1 transform in mm-group slices --------
            for (q0, sg) in mm_groups:
                a_sl = acc[:, q0 * Dh:(q0 + sg) * Dh]
                st = spool.tile([CH, MMG * Dh], f32, tag="st")
                nc.sync.dma_start(out=st[:, :sg * Dh],
                                  in_=x_own[:, q0 * Dh:(q0 + sg) * Dh])
                nc.vector.tensor_tensor(
                    out=st[:, :sg * Dh].rearrange("p (j f) -> p j f", j=sg),
                    in0=st[:, :sg * Dh].rearrange("p (j f) -> p j f", j=sg),
                    in1=dinv_o[:, q0:q0 + sg][:, :, None]
                    .to_broadcast([CH, sg, Dh]),
                    op=mybir.AluOpType.mult)
                nc.vector.tensor_tensor(out=a_sl, in0=a_sl, in1=st[:, :sg * Dh],
                                        op=mybir.AluOpType.add)
                nc.vector.tensor_tensor(
                    out=a_sl.rearrange("p (j f) -> p j f", j=sg),
                    in0=a_sl.rearrange("p (j f) -> p j f", j=sg),
                    in1=dinv_o[:, q0:q0 + sg][:, :, None]
                    .to_broadcast([CH, sg, Dh]),
                    op=mybir.AluOpType.mult)
                mm_ps = psum.tile([CH, MMG * DH], f32, tag="mmps", space="PSUM")
                tps = []
                for jj in range(sg):
                    hj = q0 + jj
                    tp_ps = psumtp.tile([CH, CH], f32, tag="tpps", space="PSUM")
                    nc.tensor.transpose(
                        out=tp_ps[:Dh, :],
                        in_=acc[:, hj * Dh:(hj + 1) * Dh],
                        identity=ident[:])
                    tp_sb = tpool.tile([CH, CH], f32, tag="tpsb")
                    nc.scalar.copy(out=tp_sb[:Dh, :], in_=tp_ps[:Dh, :])
                    tps.append(tp_sb)
                for jj in range(sg):
                    nc.tensor.matmul(
                        out=mm_ps[:, jj * DH:(jj + 1) * DH],
                        lhsT=tps[jj][:Dh, :],
                        rhs=W1T_t[:Dh, :],
                        start=True, stop=True)
                h = mpool.tile([CH, MMG * DH], f32, tag="hmm")
                nc.vector.tensor_tensor(
                    out=h[:, :sg * DH].rearrange("p (j f) -> p j f", j=sg),
                    in0=mm_ps[:, :sg * DH].rearrange("p (j f) -> p j f", j=sg),
                    in1=b1_t[:, None, :].to_broadcast([CH, sg, DH]),
                    op=mybir.AluOpType.add)
                nc.vector.tensor_scalar_max(out=h[:, :sg * DH],
                                            in0=h[:, :sg * DH], scalar1=0.0)
                nc.vector.tensor_tensor(
                    out=h[:, :sg * DH].rearrange("p (j f) -> p j f", j=sg),
                    in0=h[:, :sg * DH].rearrange("p (j f) -> p j f", j=sg),
                    in1=dinv_o[:, q0:q0 + sg][:, :, None]
                    .to_broadcast([CH, sg, DH]),
                    op=mybir.AluOpType.mult)
                nc.sync.dma_start(out=t2shard[:, q0 * DH:(q0 + sg) * DH],
                                  in_=h[:, :sg * DH])

            # ================= layer 2 (src-sharded) =================
            # gather from OWN t2shard rows, reduce per sorted-dst chunk, then
            # scatter-add into the p-major dst slab; one ReduceScatter sums
            # the 8 cores' partials and hands each core its own shard.
            t2rows = t2shard[:, :].rearrange("p (q f) -> (p q) f", f=DH)
            # scatter-add desc-gen waits on its reduce; lag it two groups
            # behind the gathers so it never head-of-line blocks the Pool SEQ.
            scat_q = []

            def stage2(b):
                cols_b = meta["cols2b"][b]
                it = ipool.tile([CH, CBW * 8], i16, tag="it", name="it")
                nc.sync.dma_start(out=it[:, :cols_b * 8],
                                  in_=ell_idx2[:, BB2[b] * 8:(BB2[b] + cols_b) * 8])
                nc.sync.dma_start(out=wt_b[b][:, :cols_b],
                                  in_=ell_w2[:, BB2[b]:BB2[b] + cols_b])
                sc = cgpool.tile([CH, PB // 16], i16, tag="sc", name="sc")
                nc.sync.dma_start(out=sc[:],
                                  in_=scat_idx[:, b * (PB // 16):(b + 1) * (PB // 16)])
                st2_b[b] = (it, sc)

            st2_b = [None] * NBUCK
            stage2(0)
            HCK = meta["NCK2"] // 2
            def scat_step(rt_, g_, q0_, b_, sc_):
                # bf16 pair rows: even-region sorted chunks write the low half
                # of each 256B row, odd-region the high half
                pv = partial2[b_ * CPB * CH:(b_ + 1) * CPB * CH, :].rearrange(
                    "a (r f) -> (a r) f", f=2 * DH)
                half = 0 if q0_ < HCK else 1
                nc.gpsimd.dma_scatter_add(
                    out_ap=pv[:, half * DH:(half + 1) * DH],
                    in_ap=rt_[:, :g_ * DH].rearrange("p (g d) -> p g d", g=g_),
                    idxs_ap=sc_[:, q0_ * 8:(q0_ + g_) * 8],
                    num_idxs=CH * g_, num_idxs_reg=CH * g_, elem_size=DH,
                    elem_step=2 * DH,
                    single_packet=False)

            for b in range(NBUCK):
                it, sc = st2_b[b]
                for g2i, (c0, cols, runs) in enumerate(meta["agg2_groups"][b]):
                    cl = c0 - BB2[b]
                    gd = gpool.tile([CH, TILE_COLS * DH], f32, tag="gd")
                    nc.gpsimd.dma_gather(
                        out_ap=gd[:, :cols * DH].rearrange("p (c d) -> p c d", c=cols),
                        in_ap=t2rows,
                        idxs_ap=it[:, cl * 8:(cl + cols) * 8],
                        num_idxs=CH * cols, num_idxs_reg=CH * cols, elem_size=DH,
                        single_packet=False)
                    if len(runs) == 1 and runs[0][2] == 1:
                        (q0, g, K, csg) = runs[0]
                        rt = rpool.tile([CH, GCAP2 * DH], bf16, tag="rt2", name="rt")
                        nc.vector.tensor_tensor(
                            out=rt[:, :g * DH].rearrange("p (c d) -> p c d", c=g),
                            in0=gd[:, :cols * DH].rearrange("p (c d) -> p c d", c=cols),
                            in1=wt_b[b][:, cl:cl + cols][:, :, None]
                            .to_broadcast([CH, cols, DH]),
                            op=mybir.AluOpType.mult)
                        scat_q.append(
                            lambda rt_=rt, g_=g, q0_=q0, b_=b, sc_=sc:
                            scat_step(rt_, g_, q0_, b_, sc_))
                        if len(scat_q) > 2:
                            scat_q.pop(0)()
                    else:
                        nc.vector.tensor_tensor(
                            out=gd[:, :cols * DH].rearrange("p (c d) -> p c d", c=cols),
                            in0=gd[:, :cols * DH].rearrange("p (c d) -> p c d", c=cols),
                            in1=wt_b[b][:, cl:cl + cols][:, :, None]
                            .to_broadcast([CH, cols, DH]),
                            op=mybir.AluOpType.mult)
                        for (q0, g, K, csg) in runs:
                            off = csg - c0
                            rt = rpool.tile([CH, GCAP2 * DH], bf16, tag="rt2", name="rt")
                            with nc.allow_low_precision("bf16 L2 partials"):
                                nc.vector.tensor_reduce(
                                    out=rt[:, :g * DH].rearrange("p (g d) -> p g d", g=g),
                                    in_=gd[:, off * DH:(off + g * K) * DH].rearrange(
                                        "p (g k d) -> p g d k", g=g, k=K),
                                    axis=mybir.AxisListType.X, op=mybir.AluOpType.add)
                            scat_q.append(
                                lambda rt_=rt, g_=g, q0_=q0, b_=b, sc_=sc:
                                scat_step(rt_, g_, q0_, b_, sc_))
                            if len(scat_q) > 2:
                                scat_q.pop(0)()
                    if g2i == 0 and b + 1 < NBUCK:
                        stage2(b + 1)
            while scat_q:
                scat_q.pop(0)()

            nc.gpsimd.collective_compute(
                "ReduceScatter", mybir.AluOpType.add,
                replica_groups=[list(range(NC))],
                ins=[partial2[:, :].opt()],
                outs=[rs_out[:, :].opt()])

            # -------- layer-2 transform --------
            for (q0, sg) in mm_groups:
                rsb = spool.tile([CH, MMG * DH], bf16, tag="rsb")
                nc.sync.dma_start(out=rsb[:, :sg * DH],
                                  in_=rs_out[:, q0 * DH:(q0 + sg) * DH])
                a_sl2 = spool.tile([CH, MMG * DH], f32, tag="a2")
                nc.vector.tensor_copy(out=a_sl2[:, :sg * DH],
                                      in_=rsb[:, :sg * DH])
                st = spool.tile([CH, MMG * DH], f32, tag="st")
                nc.sync.dma_start(out=st[:, :sg * DH],
                                  in_=t2shard[:, q0 * DH:(q0 + sg) * DH])
                nc.vector.tensor_tensor(out=a_sl2[:, :sg * DH],
                                        in0=a_sl2[:, :sg * DH],
                                        in1=st[:, :sg * DH],
                                        op=mybir.AluOpType.add)
                nc.vector.tensor_tensor(
                    out=a_sl2[:, :sg * DH].rearrange("p (j f) -> p j f", j=sg),
                    in0=a_sl2[:, :sg * DH].rearrange("p (j f) -> p j f", j=sg),
                    in1=dinv_o[:, q0:q0 + sg][:, :, None]
                    .to_broadcast([CH, sg, DH]),
                    op=mybir.AluOpType.mult)
                mm_ps = psum.tile([CH, MMG * DOUT], f32, tag="mmps2", space="PSUM")
                tps = []
                for jj in range(sg):
                    tp_ps = psumtp.tile([CH, CH], f32, tag="tpps", space="PSUM")
                    nc.tensor.transpose(
                        out=tp_ps[:DH, :],
                        in_=a_sl2[:, jj * DH:(jj + 1) * DH],
                        identity=ident[:])
                    tp_sb = tpool.tile([CH, CH], f32, tag="tpsb")
                    nc.scalar.copy(out=tp_sb[:DH, :], in_=tp_ps[:DH, :])
                    tps.append(tp_sb)
                for jj in range(sg):
                    nc.tensor.matmul(
                        out=mm_ps[:, jj * DOUT:(jj + 1) * DOUT],
                        lhsT=tps[jj][:DH, :],
                        rhs=W2T_t[:DH, :],
                        start=True, stop=True)
                h = mpool.tile([CH, MMG * DOUT], f32, tag="hmm2")
                nc.vector.tensor_tensor(
                    out=h[:, :sg * DOUT].rearrange("p (j f) -> p j f", j=sg),
                    in0=mm_ps[:, :sg * DOUT].rearrange("p (j f) -> p j f", j=sg),
                    in1=b2_t[:, None, :].to_broadcast([CH, sg, DOUT]),
                    op=mybir.AluOpType.add)
                nc.sync.dma_start(out=out[:, q0 * DOUT:(q0 + sg) * DOUT],
                                  in_=h[:, :sg * DOUT])

    nc.compile()
    _split_multi_waits(nc)
    return nc


_CACHE = {}


def kernel(x, edge_index, edge_weights, W1, b1, W2, b2):
    x = np.asarray(x); edge_index = np.asarray(edge_index)
    edge_weights = np.asarray(edge_weights)
    W1 = np.asarray(W1); b1 = np.asarray(b1)
    W2 = np.asarray(W2); b2 = np.asarray(b2)

    meta, in_maps, tpos = _prep(x, edge_index, edge_weights, W1, b1, W2, b2)
    key = (x.shape, edge_index.shape, meta["TCOLS"], meta["TCOLS2"],
           meta["LDEG"], meta["LDEGO"], meta["agg_groups"], meta["agg2_groups"],
           meta["dega_groups"], meta["degown_groups"])
    if key not in _CACHE:
        _CACHE[key] = _build(meta)
    nc = _CACHE[key]
    res = run_bass_kernel_spmd(nc, in_maps, list(range(NC)))
    NCHUNK, DOUT, SHARD = meta["NCHUNK"], meta["DOUT"], meta["SHARD"]
    blocks = [res.results[c]["out"].reshape(CH, NCHUNK, DOUT).transpose(1, 0, 2)
              .reshape(SHARD, DOUT) for c in range(NC)]
    full = np.concatenate(blocks, axis=0)
    return full[tpos].astype(np.float32)
